# revision 23
# baseline (speedup 1.0000x reference)
"""GNN mean-aggregator encoder (GraphSAGE/GCN style) on 8 Trainium2 cores.

Reference computation:
    neigh_mean = mean(features[neigh_idx], axis=1)        # [B, F]
    combined   = concat([features[nodes], neigh_mean], 1) # [B, 2F]
    out        = relu(weight @ combined.T)                # [E, B]

Sharding: data-parallel over the node batch B=50000 across 8 cores (6250
nodes each, padded to 6400 = 50 tiles of 128); weight replicated per core.

The kernel is HBM-gather bound. Levers vs the naive 17-indirect-DMAs-per-
tile version (which pays ~1 us of SWDGE fixed cost per 128 gathered rows):

  * ONE fused dma_gather per 128-node tile moves all 17*128 rows
    (2176 descriptors) in a single Pool instruction.
  * dma_gather indices are int16, so the 100k-row table cannot be indexed
    directly. The 50 tiles are split into 4 phases of <=13 tiles; each
    phase's <=28288 row references are deduplicated ON THE HOST into a
    compact per-phase table (<=32768 rows, int16-safe by construction)
    which is shipped as an input. The device still performs the full
    random 17-rows-per-node gather out of the compact tables.
  * features/weight/output in bf16: halves gathered bytes (512 B rows)
    and runs the PE at 1 cycle/column. f32 accumulation in PSUM keeps
    max-rel-err ~4e-3 (tolerance 2e-2).

Per 128-node tile on each core:
  1. one dma_gather pulls 17 rows/node into g [128, 17*256] bf16
     (row (p, slot c) = compact_table[idx[c*128+p]]).
  2. DVE tree-adds the 16 neighbor blocks -> neighbor SUM in bf16 (1/16
     mean factor pre-folded into the neighbor half of the weight on the
     host).
  3. TensorE transposes self/neigh 128x128 chunks into PSUM (combined^T),
     ACT copies to SBUF bf16.
  4. TensorE multiplies with pre-swizzled W^T chunks accumulating over the
     four 128-feature chunks -> psum [128 nodes, 256 emb] f32.
  5. ACT relu-copies to SBUF bf16, DMA to out_t [6400, 256] bf16.

Host assembles: concat core outputs' first 6250 rows, transpose, cast f32.
"""

import numpy as np

P = 128      # nodes per tile / partitions
F = 256      # feature dim
S = 16       # sampled neighbors
E = 256      # embed dim
K = 1 + S    # gathered rows per node
V = 100000   # feature table rows
B_FULL = 50000
N_CORES = 8
B_CORE = B_FULL // N_CORES          # 6250
T = (B_CORE + P - 1) // P           # 50 tiles
B_PAD = T * P                       # 6400
NI = K * P                          # 2176 gathered rows per tile
IW = NI // 16                       # 136 idx columns (16-partition wrap)
CT_ROWS = 32768                     # compact table rows per phase
# tile ranges per phase; <=13 tiles -> <=28288 refs < 32768 (int16-safe)
PHASES = [(0, 13), (13, 26), (26, 39), (39, T)]

_prog_cache = {}


def _np_bf16():
    import concourse.mybir as mybir
    return mybir.dt.np(mybir.dt.bfloat16)


def _phase_of(t):
    for i, (a, b) in enumerate(PHASES):
        if a <= t < b:
            return i
    raise ValueError(t)


def _build_program(reps=1):
    import concourse.mybir as mybir
    import concourse.tile as tile
    from concourse import bacc

    FP = mybir.dt.float32
    BF = mybir.dt.bfloat16
    nc = bacc.Bacc("TRN2", num_devices=N_CORES)

    feat_p = [nc.dram_tensor(f"feat{i}", [CT_ROWS, F], BF,
                             kind="ExternalInput")
              for i in range(len(PHASES))]
    idx_r = nc.dram_tensor("idx_r", [32, T * IW], mybir.dt.int16,
                           kind="ExternalInput")
    wt_r = nc.dram_tensor("wt_r", [P, 4 * E], BF, kind="ExternalInput")
    id_r = nc.dram_tensor("id_r", [P, P], BF, kind="ExternalInput")
    out_t = nc.dram_tensor("out_t", [B_PAD, E], BF, kind="ExternalOutput")

    with tile.TileContext(nc) as tc:
        with tc.tile_pool(name="const", bufs=1) as const, \
             tc.tile_pool(name="gpool", bufs=8) as gpool, \
             tc.tile_pool(name="wpool", bufs=6) as wpool, \
             tc.tile_pool(name="ppool", bufs=3, space="PSUM") as ppool:
            # dma_gather's queue-0 ucode reads idx only from SBUF
            # partitions 0-31 (2 replicas of the 16-partition wrap).
            # Tile 0's columns load first so gather 0 isn't gated on the
            # full index load.
            idx_sb = const.tile([P, T * IW], mybir.dt.int16)
            nc.sync.dma_start(out=idx_sb[0:32, :IW],
                              in_=idx_r.ap()[:, :IW])
            nc.sync.dma_start(out=idx_sb[0:32, IW:],
                              in_=idx_r.ap()[:, IW:])
            wt_sb = const.tile([P, 4 * E], BF)
            nc.sync.dma_start(out=wt_sb[:], in_=wt_r.ap())
            # identity from DRAM keeps gpsimd memset/affine_select off
            # the Pool queue ahead of gather 0's descriptor generation
            ident = const.tile([P, P], BF)
            nc.sync.dma_start(out=ident[:], in_=id_r.ap())

            for t in [tt for _ in range(reps) for tt in range(T)]:
                g = gpool.tile([P, K * F], BF, tag="g")
                last = (t == T - 1)
                base = t * IW
                if t == 0:
                    # smaller first gather -> shorter desc-gen on the
                    # startup critical path; slots land identically so the
                    # normal tree below just depends on both halves
                    nc.gpsimd.dma_gather(
                        g[:, :8 * F].rearrange("p (c e) -> p c e", e=F),
                        feat_p[0].ap(),
                        idx_sb[0:32, base:base + 64],
                        8 * P, 8 * P, F, single_packet=False)
                    nc.gpsimd.dma_gather(
                        g[:, 8 * F:].rearrange("p (c e) -> p c e", e=F),
                        feat_p[0].ap(),
                        idx_sb[0:32, base + 64:base + IW],
                        9 * P, 9 * P, F, single_packet=False)
                elif not last:
                    nc.gpsimd.dma_gather(
                        g[:].rearrange("p (c e) -> p c e", e=F),
                        feat_p[_phase_of(t)].ap(),
                        idx_sb[0:32, base:base + IW],
                        NI, NI, F, single_packet=False)
                else:
                    # split the final gather so most of the drain-tail
                    # compute overlaps the second half-gather
                    nc.gpsimd.dma_gather(
                        g[:, :8 * F].rearrange("p (c e) -> p c e", e=F),
                        feat_p[_phase_of(t)].ap(),
                        idx_sb[0:32, base:base + 64],
                        8 * P, 8 * P, F, single_packet=False)
                    nc.gpsimd.dma_gather(
                        g[:, 8 * F:].rearrange("p (c e) -> p c e", e=F),
                        feat_p[_phase_of(t)].ap(),
                        idx_sb[0:32, base + 64:base + IW],
                        9 * P, 9 * P, F, single_packet=False)
                ct_ps = ppool.tile([P, 4 * P], BF, tag="ct")
                for c in range(2):
                    nc.tensor.transpose(ct_ps[:, c * P:(c + 1) * P],
                                        g[:, c * P:(c + 1) * P], ident[:])
                # neighbor sum: bf16 tree of adds on the (otherwise idle) DVE
                nsum = wpool.tile([P, F], BF, tag="nsum")
                if not last:
                    n1 = wpool.tile([P, 8 * F], BF, tag="n1")
                    nc.vector.tensor_tensor(n1[:], g[:, F:9 * F],
                                            g[:, 9 * F:17 * F],
                                            mybir.AluOpType.add)
                    n2 = wpool.tile([P, 4 * F], BF, tag="n2")
                    nc.vector.tensor_tensor(n2[:], n1[:, :4 * F],
                                            n1[:, 4 * F:],
                                            mybir.AluOpType.add)
                    n3 = wpool.tile([P, 2 * F], BF, tag="n3")
                    nc.vector.tensor_tensor(n3[:], n2[:, :2 * F],
                                            n2[:, 2 * F:],
                                            mybir.AluOpType.add)
                    nc.vector.tensor_tensor(nsum[:], n3[:, :F], n3[:, F:],
                                            mybir.AluOpType.add)
                else:
                    # A-part (slots 1-7, available after half-gather A)
                    a1 = wpool.tile([P, 3 * F], BF, tag="n1")
                    nc.vector.tensor_tensor(a1[:], g[:, F:4 * F],
                                            g[:, 4 * F:7 * F],
                                            mybir.AluOpType.add)
                    a2 = wpool.tile([P, F], BF, tag="n2")
                    nc.vector.tensor_tensor(a2[:], a1[:, :F], a1[:, F:2 * F],
                                            mybir.AluOpType.add)
                    a3 = wpool.tile([P, F], BF, tag="n3")
                    nc.vector.tensor_tensor(a3[:], a2[:], a1[:, 2 * F:],
                                            mybir.AluOpType.add)
                    a4 = wpool.tile([P, F], BF, tag="n4")
                    nc.vector.tensor_tensor(a4[:], a3[:], g[:, 7 * F:8 * F],
                                            mybir.AluOpType.add)
                    # B-part (slots 8-16, after half-gather B)
                    b1 = wpool.tile([P, 4 * F], BF, tag="n5")
                    nc.vector.tensor_tensor(b1[:], g[:, 8 * F:12 * F],
                                            g[:, 12 * F:16 * F],
                                            mybir.AluOpType.add)
                    b2 = wpool.tile([P, 2 * F], BF, tag="n6")
                    nc.vector.tensor_tensor(b2[:], b1[:, :2 * F],
                                            b1[:, 2 * F:],
                                            mybir.AluOpType.add)
                    b3 = wpool.tile([P, F], BF, tag="n7")
                    nc.vector.tensor_tensor(b3[:], b2[:, :F], b2[:, F:],
                                            mybir.AluOpType.add)
                    b4 = wpool.tile([P, F], BF, tag="n8")
                    nc.vector.tensor_tensor(b4[:], b3[:], g[:, 16 * F:],
                                            mybir.AluOpType.add)
                    nc.vector.tensor_tensor(nsum[:], a4[:], b4[:],
                                            mybir.AluOpType.add)
                for c in range(2):
                    nc.tensor.transpose(ct_ps[:, (2 + c) * P:(3 + c) * P],
                                        nsum[:, c * P:(c + 1) * P], ident[:])
                ct = wpool.tile([P, 4 * P], BF, tag="ct_sb")
                nc.scalar.activation(ct[:, :2 * P], ct_ps[:, :2 * P],
                                     mybir.ActivationFunctionType.Copy)
                nc.scalar.activation(ct[:, 2 * P:], ct_ps[:, 2 * P:],
                                     mybir.ActivationFunctionType.Copy)
                psum_o = ppool.tile([P, E], FP, tag="po")
                for c in range(4):
                    nc.tensor.matmul(
                        psum_o[:], lhsT=ct[:, c * P:(c + 1) * P],
                        rhs=wt_sb[:, c * E:(c + 1) * E],
                        start=(c == 0), stop=(c == 3))
                ot = wpool.tile([P, E], BF, tag="ot")
                nc.scalar.activation(ot[:], psum_o[:],
                                     mybir.ActivationFunctionType.Relu)
                nc.sync.dma_start(out=out_t.ap()[t * P:(t + 1) * P, :],
                                  in_=ot[:])
    nc.compile()
    return nc


def get_program(reps=1):
    key = ("nc", reps)
    if key not in _prog_cache:
        _prog_cache[key] = _build_program(reps)
    return _prog_cache[key]


def _prep_core(nodes_c, neigh_c, feat_bf):
    """Per-core indices + bf16 feature table -> (idx_r [P, T*IW] int16,
    [compact table per phase])."""
    b = nodes_c.shape[0]
    idx_all = np.zeros((B_PAD, K), np.int64)
    idx_all[:b, 0] = nodes_c
    idx_all[:b, 1:] = neigh_c

    local = np.empty((B_PAD, K), np.int16)
    tables = []
    for (a, bb) in PHASES:
        refs = idx_all[a * P:bb * P]                  # [(bb-a)*128, K]
        uniq, inv = np.unique(refs, return_inverse=True)
        assert len(uniq) <= CT_ROWS, len(uniq)
        local[a * P:bb * P] = inv.reshape(refs.shape).astype(np.int16)
        tbl = np.zeros((CT_ROWS, F), feat_bf.dtype)
        tbl[:len(uniq)] = feat_bf[uniq]
        tables.append(tbl)

    # per tile: list position c*128+p = local[t*128+p, c]; wrap into 16
    # partitions (pos i -> [i%16, i//16]); queue-0 dma_gather reads the
    # wrap from SBUF partitions 0-31, so upload 2 replicas only.
    lt = local.reshape(T, P, K)
    flat = lt.transpose(0, 2, 1).reshape(T, NI)       # [T, 2176]
    wrapped = flat.reshape(T, IW, 16).transpose(0, 2, 1)   # [T, 16, IW]
    # last tile is gathered in two halves (slots 0-7, then 8-16): its
    # column block holds the two sub-lists wrapped independently.
    for tt in (0, T - 1):
        la = lt[tt, :, :8].T.ravel()                  # [1024]
        lb = lt[tt, :, 8:].T.ravel()                  # [1152]
        wrapped[tt, :, :64] = la.reshape(64, 16).T
        wrapped[tt, :, 64:] = lb.reshape(72, 16).T
    rep = np.tile(wrapped, (1, 2, 1))                 # [T, 32, IW]
    idx_r = np.ascontiguousarray(
        rep.transpose(1, 0, 2).reshape(32, T * IW))
    return idx_r, tables


def _prep_weight(weight):
    """[E, 2F] -> chunk-swizzled W.T [P, 4*E] f32 with mean pre-folded."""
    wt = np.asarray(weight, dtype=np.float32).T.copy()   # [2F, E]
    wt[F:] /= S
    return np.ascontiguousarray(
        wt.reshape(4, P, E).transpose(1, 0, 2).reshape(P, 4 * E)
    ).astype(_np_bf16())


def make_in_maps(nodes, neigh_idx, features, weight):
    nodes = np.asarray(nodes)
    neigh_idx = np.asarray(neigh_idx)
    feat_bf = np.ascontiguousarray(
        np.asarray(features, dtype=np.float32)).astype(_np_bf16())
    wt_r = _prep_weight(weight)
    id_np = np.eye(P, dtype=np.float32).astype(_np_bf16())
    in_maps = []
    for c in range(N_CORES):
        sl = slice(c * B_CORE, (c + 1) * B_CORE)
        idx_r, tables = _prep_core(nodes[sl], neigh_idx[sl], feat_bf)
        m = {"idx_r": idx_r, "wt_r": wt_r, "id_r": id_np}
        for i, tbl in enumerate(tables):
            m[f"feat{i}"] = tbl
        in_maps.append(m)
    return in_maps


def kernel(nodes, neigh_idx, features, weight):
    import concourse.bass_utils as bass_utils

    assert np.asarray(nodes).shape[0] == B_FULL, "kernel hardcodes B=50000"
    nc = get_program()
    in_maps = make_in_maps(nodes, neigh_idx, features, weight)
    res = bass_utils.run_bass_kernel_spmd(
        nc, in_maps, core_ids=list(range(N_CORES)))
    out_t = np.concatenate(
        [np.asarray(res.results[c]["out_t"][:B_CORE], dtype=np.float32)
         for c in range(N_CORES)], axis=0)
    return np.ascontiguousarray(out_t.T)
